# revision 57
# baseline (speedup 1.0000x reference)
"""Trainium2 Bass kernel for nn_CAM_41377714929724 (CAM cross-attention module).

  a1  = f1 @ W                      [B,S,D]
  cc  = a1 @ f2^T                   [B,S,S]
  aatt = softmax(cc, axis=s)        (over rows -> column-normalized)
  vatt = softmax(cc, axis=t).T      (over cols, transposed)
  out1 = (f1 @ aatt).swap(1,2)      [B,S,S]
  out2 = (f2 @ vatt).swap(1,2)      [B,S,S]

Sharding: pure data parallelism, 2 batches per core on 8 cores; W replicated.

PE runs only the four 1024^3 GEMMs per batch (G1/G2 f32r for the precision-
sensitive logit chain, G3/G4 bf16 post-softmax; bf16 operand rounding after
the softmax adds <0.1% output error) plus cheap bf16 transpose-mode matmuls.
Everything else lives on DVE/ACT/gpsimd/DMA:
 - ~6us HAM warm-up burst on the identity tile so the PE clock gate reaches
   K=8/8 before G1's load-paced matmuls begin
 - G1 runs k-outer in passes of 3/3/2 m-tiles so its matmuls pace with the
   per-k-tile input DMAs instead of waiting for the full 8MB
 - each f32r GEMM group runs its two 512-wide halves k-inner so both share
   every (forced, self-loaded) LDWEIGHTS; drains split DVE/ACT in parallel
 - e2 = exp(cc - rowmax) on ACT with per-partition bias; the same op emits
   vsum via accum_out; e2T built by PE transpose-mode (bf16, banks 6/7),
   lagged one m-tile so the PE never waits on the exp's cross-engine latency
   (the DMA xbar alternative serializes at ~3.5us/call - far too slow)
 - cc spilled to DRAM (scalar queue), reloaded (sync queue) for
   e1 = exp(cc - colmax); colmax via DVE running max + gpsimd
   partition_all_reduce (library preloaded via a dummy op at t=0)
 - asum -> 1/asum per-partition scale via DRAM bounce ([128,8] reciprocal)
 - queue discipline: sync = loads/reloads/bounces, scalar = spills/stores/
   drains, gpsimd = late bf16 loads + all-reduces, ordered so no DMA issue
   ever head-of-line-blocks a latency-critical op
 - per-batch softmax work is woven instruction-by-instruction into the other
   batch's PE phases so every engine FIFO stays dependency-clean

PE order: b0.G1 b0.G2 | b1.G1(+b0 softmax) | b0.G4 b0.G3[0:5] | b1.G2 |
          b0.G3[5:8]+b1.G4 (+b1 softmax woven 2/group) | b1.G3
Main GEMMs rotate PSUM banks 0-5; transposes/warm-up own banks 6-7.
Measured: 285430 ns (baseline 326967 ns), rel err 2.1e-3.
"""

import numpy as np
import ml_dtypes
from contextlib import ExitStack

import concourse.bass as bass
import concourse.tile as tile
from concourse import bacc, mybir, bass_isa
from concourse.bass_utils import run_bass_kernel_spmd

f32 = mybir.dt.float32
f32r = mybir.dt.float32r
bf16 = mybir.dt.bfloat16

P = 128
N = 1024
NT = N // P
NB = 2
NCORES = 8
HALF = 512
BIG = NT * N
Exp = mybir.ActivationFunctionType.Exp
Copy = mybir.ActivationFunctionType.Copy


def _build():
    nc = bacc.Bacc("TRN2", target_bir_lowering=False, debug=False, num_devices=NCORES)

    f1t_d = nc.dram_tensor("f1t", [NB, N, N], f32r, kind="ExternalInput").ap()
    f2t_d = nc.dram_tensor("f2t", [NB, N, N], f32r, kind="ExternalInput").ap()
    f1b_d = nc.dram_tensor("f1b", [NB, N, N], bf16, kind="ExternalInput").ap()
    f2b_d = nc.dram_tensor("f2b", [NB, N, N], bf16, kind="ExternalInput").ap()
    w_d = nc.dram_tensor("w", [N, N], f32r, kind="ExternalInput").ap()
    identb_d = nc.dram_tensor("identb", [P, P], bf16, kind="ExternalInput").ap()
    o1_d = nc.dram_tensor("o1", [NB, N, N], f32, kind="ExternalOutput").ap()
    o2_d = nc.dram_tensor("o2", [NB, N, N], f32, kind="ExternalOutput").ap()

    with tile.TileContext(nc) as tc, ExitStack() as ctx:
        kp = ctx.enter_context(tc.tile_pool(name="kp", bufs=1))
        bigp = ctx.enter_context(tc.tile_pool(name="bigp", bufs=1))
        ccp = ctx.enter_context(tc.tile_pool(name="ccp", bufs=3))
        e2p = ctx.enter_context(tc.tile_pool(name="e2p", bufs=3))
        ostp = ctx.enter_context(tc.tile_pool(name="ostp", bufs=2))
        ost2p = ctx.enter_context(tc.tile_pool(name="ost2p", bufs=2))
        statp = ctx.enter_context(tc.tile_pool(name="statp", bufs=1))
        stat2p = ctx.enter_context(tc.tile_pool(name="stat2p", bufs=2))
        psp = ctx.enter_context(tc.tile_pool(name="psp", bufs=1, space="PSUM"))
        dscrp = ctx.enter_context(tc.tile_pool(name="dscrp", bufs=2, space="DRAM"))

        def ktiles(pfx, tag_pfx, dt, width=N):
            return [kp.tile([P, width], dt, name=f"{pfx}{k}", tag=f"{tag_pfx}{k}")
                    for k in range(NT)]

        wts = ktiles("w", "w", f32r)
        f1s_0 = ktiles("f1_0", "f1", f32r)
        identb = kp.tile([P, P], bf16, name="identb", tag="identb")
        nc.sync.dma_start(identb[:], identb_d[:, :])
        # first k-tiles split into quarters: the opening matmuls then wait
        # ~128KB per queue instead of a full 512KB transfer
        for k in range(NT):
            nsplit = 4 if k < 4 else 1
            w = N // nsplit
            for s in range(nsplit):
                nc.sync.dma_start(
                    wts[k][:, s * w:(s + 1) * w],
                    w_d[k * P:(k + 1) * P, s * w:(s + 1) * w])
                nc.sync.dma_start(
                    f1s_0[k][:, s * w:(s + 1) * w],
                    f1t_d[0, k * P:(k + 1) * P, s * w:(s + 1) * w])

        # HAM warm-up: ~6us of dummy matmuls on the identity tile while the
        # first input tiles stream in. Without this the clock gate stays at
        # K=4/8 (1.2GHz) deep into G1 because the load-paced matmuls are too
        # sparse to trip the activity monitor.
        ps_warm = psp.tile([P, P], f32, name="ps_warm", tag="ps6")
        for i in range(64):
            nc.tensor.matmul(ps_warm[:], identb[:], identb[:],
                             start=(i == 0), stop=(i == 63))

        a1s = ktiles("a1", "a1", f32r)
        per_b = []
        for b in range(NB):
            d = {}
            d["ccsp"] = dscrp.tile([P, BIG], f32, name=f"ccsp{b}", tag="ccsp")
            d["scr"] = dscrp.tile([1, N], f32, name=f"scr{b}", tag="scr")
            d["nvmax"] = stat2p.tile([P, NT], f32, name=f"nvmax{b}", tag="nvmax")
            d["vs"] = stat2p.tile([P, NT], f32, name=f"vs{b}", tag="vs")
            d["rv"] = stat2p.tile([P, NT], f32, name=f"rv{b}", tag="rv")
            d["rsa"] = stat2p.tile([P, NT], f32, name=f"rsa{b}", tag="rsa")
            per_b.append(d)

        # ---- GEMM group: k-inner, both 512-halves share each LDWEIGHTS ----
        # drain halves go to DVE (n=0) and ACT (n=1) in parallel.
        def gemm(lhsT_sl, rhs_sl, m, drain0, drain1, pstag, psbase=None):
            pst = (m % 3) * 2 if psbase is None else psbase
            ps0 = psp.tile([P, HALF], f32, name=f"ps_{pstag}_{m}_0",
                           tag=f"ps{pst}")
            ps1 = psp.tile([P, HALF], f32, name=f"ps_{pstag}_{m}_1",
                           tag=f"ps{pst + 1}")
            for k in range(NT):
                nc.tensor.matmul(ps0[:], lhsT_sl(k, m), rhs_sl(k, 0),
                                 start=(k == 0), stop=(k == NT - 1))
                nc.tensor.matmul(ps1[:], lhsT_sl(k, m), rhs_sl(k, 1),
                                 start=(k == 0), stop=(k == NT - 1))
            drain0(m, 0, ps0)
            drain1(m, 1, ps1)

        def sl_k(tiles):
            return lambda k, m: tiles[k][:, m * P:(m + 1) * P]

        def sl_kr(tiles, base=0):
            return lambda k, n: tiles[k][:, base + n * HALF: base + (n + 1) * HALF]

        def sl_big(t):
            return lambda k, m: t[:, k * N + m * P: k * N + (m + 1) * P]

        state = {}

        # ---------------- G1: k-outer passes, paces with per-k loads -------
        # passes of 3/3/2 m-tiles use only PSUM banks 0-5, leaving 6/7 to
        # the e2 transposes that run concurrently during G2 phases
        def g1(b, f1_tiles, after_pass0=None):
            for mlo, mhi in ((0, 3), (3, 6), (6, 8)):
                if mlo == 3 and after_pass0 is not None:
                    after_pass0()
                pss = []
                for mi, m in enumerate(range(mlo, mhi)):
                    pss.append((
                        psp.tile([P, HALF], f32, name=f"psg1_{b}_{m}_0",
                                 tag=f"ps{mi * 2}"),
                        psp.tile([P, HALF], f32, name=f"psg1_{b}_{m}_1",
                                 tag=f"ps{mi * 2 + 1}")))
                for k in range(NT):
                    for mi, m in enumerate(range(mlo, mhi)):
                        lh = wts[k][:, m * P:(m + 1) * P]
                        nc.tensor.matmul(pss[mi][0][:], lh,
                                         f1_tiles[k][:, 0:HALF],
                                         start=(k == 0), stop=(k == NT - 1))
                        nc.tensor.matmul(pss[mi][1][:], lh,
                                         f1_tiles[k][:, HALF:N],
                                         start=(k == 0), stop=(k == NT - 1))
                for mi, m in enumerate(range(mlo, mhi)):
                    nc.vector.tensor_copy(a1s[m][:, 0:HALF], pss[mi][0][:])
                    nc.scalar.copy(a1s[m][:, HALF:N], pss[mi][1][:])

        # ---------------- G2 + per-m softmax-2 (e2) path -------------------
        # e2T is built by PE transpose-mode matmuls (bf16, 1 cyc/row) into
        # the reserved PSUM banks 6/7, lagged one m-tile behind the GEMM so
        # the PE never waits on the exp's cross-engine latency. g2 returns a
        # closure emitting the last tile's transposes, which the caller
        # places after the next phase's first PE group.
        def transpose_tile(b, m, e2t, e2t_t):
            e2tv = e2t_t[:, :].rearrange("p (j c) -> p j c", c=N)
            for j0 in range(2):
                psT = psp.tile([P, HALF], bf16, name=f"psT_{b}_{m}_{j0}",
                               tag=f"ps{6 + j0}")
                for q in range(4):
                    j = 4 * j0 + q
                    nc.tensor.matmul(psT[:, q * P:(q + 1) * P],
                                     e2t[:, j * P:(j + 1) * P], identb[:],
                                     is_transpose=True,
                                     start=(q == 0), stop=(q == 3))
                dr = nc.vector.tensor_copy if j0 == 0 else nc.scalar.copy
                dr(e2tv[:, 4 * j0:4 * j0 + 4, m * P:(m + 1) * P],
                   psT[:, :].rearrange("p (j c) -> p j c", c=P))

        def g2(b, f2_tiles, e2t_t):
            d = per_b[b]
            amaxacc = statp.tile([P, N], f32, name=f"amaxacc{b}", tag="amaxacc")
            ccs = []
            e2s = []

            def drain0(m, n, ps):
                nc.vector.tensor_copy(ccs[m][:, 0:HALF], ps[:])

            def drain1(m, n, ps):
                nc.scalar.copy(ccs[m][:, HALF:N], ps[:])

            for m in range(NT):
                cct = ccp.tile([P, N], f32, name=f"cc_{b}_{m}", tag="cc")
                ccs.append(cct)
                gemm(sl_k(a1s), sl_kr(f2_tiles), m, drain0, drain1, f"cc_{b}")
                if m > 0:
                    transpose_tile(b, m - 1, e2s[m - 1], e2t_t)
                nc.scalar.dma_start(d["ccsp"][:, m * N:(m + 1) * N], cct[:])
                nc.vector.tensor_reduce(
                    out=d["nvmax"][:, m:m + 1], in_=cct[:],
                    axis=mybir.AxisListType.X, op=mybir.AluOpType.max,
                    negate=True)
                e2t = e2p.tile([P, N], bf16, name=f"e2_{b}_{m}", tag="e2")
                e2s.append(e2t)
                # exp with per-partition bias; accum_out = row sum = vsum
                nc.scalar.activation(e2t[:], cct[:], Exp,
                                     bias=d["nvmax"][:, m:m + 1],
                                     accum_out=d["vs"][:, m:m + 1])
                if m == 0:
                    nc.vector.tensor_copy(amaxacc[:], cct[:])
                else:
                    nc.vector.tensor_tensor(
                        out=amaxacc[:], in0=amaxacc[:], in1=cct[:],
                        op=mybir.AluOpType.max)
            nc.vector.reciprocal(d["rv"][:], d["vs"][:])
            state[b] = dict(amaxacc=amaxacc)
            return lambda: transpose_tile(b, NT - 1, e2s[NT - 1], e2t_t)

        # ---------------- softmax-1 (e1) path, emitted piecewise -----------
        def sm1_start(b):
            d = per_b[b]
            amaxB = statp.tile([P, N], f32, name=f"amaxB{b}", tag="amaxB")
            nc.gpsimd.partition_all_reduce(
                amaxB[:], state[b]["amaxacc"][:], channels=P,
                reduce_op=bass_isa.ReduceOp.max)
            asumacc = statp.tile([P, N], f32, name=f"asumacc{b}", tag="asumacc")
            state[b].update(amaxB=amaxB, asumacc=asumacc)

        def _sm1_add(b, m, combo_tiles):
            st = state[b]
            if m == 0:
                nc.vector.tensor_copy(st["asumacc"][:], combo_tiles[m][:, 0:N])
            else:
                nc.vector.tensor_tensor(
                    out=st["asumacc"][:], in0=st["asumacc"][:],
                    in1=combo_tiles[m][:, 0:N], op=mybir.AluOpType.add)

        def sm1_piece(b, m, combo_tiles):
            """reload+sub+exp for tile m; the asum add lags one tile so the
            DVE never queue-blocks on this tile's ACT exp."""
            d = per_b[b]
            st = state[b]
            ccr = ccp.tile([P, N], f32, name=f"ccr_{b}_{m}", tag="cc")
            nc.sync.dma_start(ccr[:], d["ccsp"][:, m * N:(m + 1) * N])
            nc.vector.tensor_tensor(
                out=ccr[:], in0=ccr[:], in1=st["amaxB"][:],
                op=mybir.AluOpType.subtract)
            nc.scalar.activation(combo_tiles[m][:, 0:N], ccr[:], Exp)
            if m > 0:
                _sm1_add(b, m - 1, combo_tiles)

        def sm1_finish(b, combo_tiles):
            d = per_b[b]
            st = state[b]
            _sm1_add(b, NT - 1, combo_tiles)
            asumB = statp.tile([P, N], f32, name=f"asumB{b}", tag="amaxB")
            nc.gpsimd.partition_all_reduce(
                asumB[:], st["asumacc"][:], channels=P,
                reduce_op=bass_isa.ReduceOp.add)
            nc.sync.dma_start(d["scr"][0:1, :], asumB[0:1, :])
            nc.sync.dma_start(
                d["rsa"][:],
                d["scr"][0:1, :].rearrange("one (m p) -> (one p) m", p=P))
            nc.vector.reciprocal(d["rsa"][:], d["rsa"][:])

        # ---------------- output GEMMs -------------------------------------
        def g4_group(b, e2t_t, f2b_tiles, m):
            d = per_b[b]
            ost = ostp.tile([P, N], f32, name=f"ost4_{b}", tag="ost")

            def drain0(m_, n, ps):
                nc.vector.tensor_scalar_mul(
                    ost[:, 0:HALF], ps[:], d["rv"][:, m_:m_ + 1])

            def drain1(m_, n, ps):
                nc.scalar.activation(ost[:, HALF:N], ps[:], Copy,
                                     bias=0.0, scale=d["rv"][:, m_:m_ + 1])
            gemm(sl_big(e2t_t), sl_kr(f2b_tiles), m, drain0, drain1,
                 f"r2_{b}")
            nc.scalar.dma_start(o2_d[b, m * P:(m + 1) * P, :], ost[:])

        def g3_group(b, combo_tiles, m, psbase=None, last=False):
            d = per_b[b]
            ost = ost2p.tile([P, N], f32, name=f"ost3_{b}", tag="ost2")

            def drain0(m_, n, ps):
                nc.vector.tensor_scalar_mul(
                    ost[:, 0:HALF], ps[:], d["rsa"][:, m_:m_ + 1])

            def drain1(m_, n, ps):
                nc.scalar.activation(ost[:, HALF:N], ps[:], Copy,
                                     bias=0.0, scale=d["rsa"][:, m_:m_ + 1])
            gemm(sl_k(combo_tiles), sl_kr(combo_tiles, base=N), m,
                 drain0, drain1, f"r1_{b}", psbase=psbase)
            if last:
                nc.sync.dma_start(
                    o1_d[b, m * P:(m + 1) * P, 0:HALF], ost[:, 0:HALF])
                nc.scalar.dma_start(
                    o1_d[b, m * P:(m + 1) * P, HALF:N], ost[:, HALF:N])
            else:
                nc.scalar.dma_start(o1_d[b, m * P:(m + 1) * P, :], ost[:])

        # ================= global schedule =================================
        # dummy gpsimd custom op: forces the Pool LOAD_LIB during idle P1
        # instead of on the first latency-critical all-reduce
        dummy = stat2p.tile([P, 1], f32, name="dummy", tag="dummy")
        nc.vector.memset(dummy[:], 0.0)
        nc.gpsimd.partition_all_reduce(dummy[:], dummy[:], channels=P,
                                       reduce_op=bass_isa.ReduceOp.max)

        f2s_0 = ktiles("f2_0", "f2", f32r)
        f2bs_0 = ktiles("f2b_0", "f2b", bf16)
        for k in range(NT):
            nc.sync.dma_start(f2s_0[k][:], f2t_d[0, k * P:(k + 1) * P, :])
            nc.sync.dma_start(f2bs_0[k][:], f2b_d[0, k * P:(k + 1) * P, :])

        # P1: b0.G1
        g1(0, f1s_0)

        f1s_1 = [kp.tile([P, N], f32r, name=f"f1_1{k}", tag=f"f1{k}")
                 for k in range(NT)]
        for k in range(NT):
            nc.sync.dma_start(f1s_1[k][:], f1t_d[1, k * P:(k + 1) * P, :])

        # P2: b0.G2
        e2t_0 = bigp.tile([P, BIG], bf16, name="e2t_0", tag="e2t")
        tr_last_0 = g2(0, f2s_0, e2t_0)

        combo_0 = [kp.tile([P, 2 * N], bf16, name=f"combo_0{k}", tag=f"f2{k}")
                   for k in range(NT)]

        # P3: b1.G1 with b0's softmax-1 pieces around it (subs before the
        # G1 drains hit the DVE FIFO, rest after). The m7 transposes of b0
        # slot in behind b1.G1's first MMs so they never stall the PE on
        # the exp latency.
        sm1_start(0)
        for m in range(0, 3):
            sm1_piece(0, m, combo_0)
        g1(1, f1s_1, after_pass0=tr_last_0)
        for m in range(3, NT):
            sm1_piece(0, m, combo_0)
        # f1b half of combo_0 loads after the e1 exps (same-tile writers);
        # gpsimd queue so the sync load stream stays clean
        for k in range(NT):
            nc.gpsimd.dma_start(combo_0[k][:, N:2 * N],
                                f1b_d[0, k * P:(k + 1) * P, :])

        f2s_1 = [kp.tile([P, N], f32r, name=f"f2_1{k}", tag=f"f1{k}")
                 for k in range(NT)]
        for k in range(NT):
            nc.sync.dma_start(f2s_1[k][:], f2t_d[1, k * P:(k + 1) * P, :])
        f2bs_1 = ktiles("f2b_1", "f2b", bf16)
        for k in range(NT):
            nc.sync.dma_start(f2bs_1[k][:], f2b_d[1, k * P:(k + 1) * P, :])

        # P4: b0.G4; sm1_finish after it so the rsa reciprocal sits behind
        # the G4 drains in the DVE FIFO (rsa is only needed at P5)
        for m in range(NT):
            g4_group(0, e2t_0, f2bs_0, m)
        sm1_finish(0, combo_0)

        # combo_1's f1b half loads now: its WAR (f1_1 readers) cleared at
        # P3-end, and it must not queue behind b1's amax all-reduce on
        # gpsimd (which can only run after P6)
        combo_1 = [kp.tile([P, 2 * N], bf16, name=f"combo_1{k}", tag=f"w{k}")
                   for k in range(NT)]
        for k in range(NT):
            nc.gpsimd.dma_start(combo_1[k][:, N:2 * N],
                                f1b_d[1, k * P:(k + 1) * P, :])

        # P5: b0.G3 first 5 m-tiles
        for m in range(0, 5):
            g3_group(0, combo_0, m)

        # P6: b1.G2
        e2t_1 = bigp.tile([P, BIG], bf16, name="e2t_1", tag="e2t")
        tr_last_1 = g2(1, f2s_1, e2t_1)

        # P7: b0.G3 tail + b1.G4, with b1's softmax-1 woven per-group;
        # b1's m7 transposes slot in behind the first tail group
        sm1_start(1)
        weave = iter(range(NT))
        first_tail = True
        for m in range(5, NT):
            g3_group(0, combo_0, m)
            if first_tail:
                tr_last_1()
                first_tail = False
            for wm in (next(weave, None), next(weave, None)):
                if wm is not None:
                    sm1_piece(1, wm, combo_1)
        for m in range(NT):
            g4_group(1, e2t_1, f2bs_1, m)
            wm = next(weave, None)
            if wm is not None:
                sm1_piece(1, wm, combo_1)
        sm1_finish(1, combo_1)

        # P8: b1.G3 (last group's store split across both hwdge queues to
        # shorten the end-of-kernel drain tail)
        for m in range(NT):
            g3_group(1, combo_1, m, last=(m == NT - 1))

    nc.compile()
    return nc


_NC = None
TRACE = False
LAST = None


def _get_nc():
    global _NC
    if _NC is None:
        _NC = _build()
    return _NC


def kernel(f1_norm, f2_norm, corr_weights):
    f1_norm = np.ascontiguousarray(f1_norm, dtype=np.float32)
    f2_norm = np.ascontiguousarray(f2_norm, dtype=np.float32)
    w = np.ascontiguousarray(corr_weights, dtype=np.float32)
    B = f1_norm.shape[0]
    assert B == NB * NCORES

    f1t = np.ascontiguousarray(np.swapaxes(f1_norm, 1, 2))
    f2t = np.ascontiguousarray(np.swapaxes(f2_norm, 1, 2))
    f1b = f1t.astype(ml_dtypes.bfloat16)
    f2b = f2t.astype(ml_dtypes.bfloat16)
    identb = np.eye(P, dtype=ml_dtypes.bfloat16)

    nc = _get_nc()
    in_maps = [
        {"f1t": f1t[c * NB:(c + 1) * NB], "f2t": f2t[c * NB:(c + 1) * NB],
         "f1b": f1b[c * NB:(c + 1) * NB], "f2b": f2b[c * NB:(c + 1) * NB],
         "w": w, "identb": identb}
        for c in range(NCORES)
    ]
    res = run_bass_kernel_spmd(nc, in_maps, core_ids=list(range(NCORES)), trace=TRACE)
    global LAST
    LAST = res
    out1 = np.concatenate([res.results[c]["o1"] for c in range(NCORES)], axis=0)
    out2 = np.concatenate([res.results[c]["o2"] for c in range(NCORES)], axis=0)
    return out1, out2


# revision 61
# speedup vs baseline: 1.0509x; 1.0509x over previous
"""Trainium2 Bass kernel for nn_CAM_41377714929724 (CAM cross-attention module).

  a1  = f1 @ W                      [B,S,D]
  cc  = a1 @ f2^T                   [B,S,S]
  aatt = softmax(cc, axis=s)        (over rows -> column-normalized)
  vatt = softmax(cc, axis=t).T      (over cols, transposed)
  out1 = (f1 @ aatt).swap(1,2)      [B,S,S]
  out2 = (f2 @ vatt).swap(1,2)      [B,S,S]

Sharding: pure data parallelism, 2 batches per core on 8 cores; W replicated.

PE runs only the four 1024^3 GEMMs per batch (G1/G2 f32r for the precision-
sensitive logit chain, G3/G4 bf16 post-softmax) plus cheap bf16 transpose-
mode matmuls and a ~6us HAM warm-up burst. Everything else lives on
DVE/ACT/gpsimd/DMA:
 - G1 runs k-outer in passes of 3/3/2 m-tiles so its matmuls pace with the
   per-k-tile input DMAs; each f32r GEMM group runs its two 512-wide halves
   k-inner so both share every (forced, self-loaded) LDWEIGHTS
 - e2 = exp(cc - rowmax) via ACT with per-partition bias; the same ACT op
   emits vsum through accum_out (free-dim sum) -> no separate reduce
 - e2T via PE transpose-mode (bf16, PSUM banks 6/7), lagged one m-tile so
   the PE never waits on the exp's cross-engine latency (the DMA xbar
   alternative serializes at ~3.5us per [128,512] call - far too slow)
 - cc spilled to DRAM (scalar queue), reloaded (sync) for e1 = exp(cc -
   colmax); colmax via DVE running max + gpsimd partition_all_reduce
   (library preloaded by a dummy op at t=0)
 - asum -> 1/asum scale vector via DRAM bounce (tiny [128,8] reciprocal)
 - every GEMM group drains its two 512-halves on DVE and ACT in parallel;
   queue discipline keeps sync = loads/reloads/bounces, scalar = spills/
   stores/drains, gpsimd = late bf16 loads + all-reduces
 - per-batch softmax work is woven instruction-by-instruction into the other
   batch's PE filler loops so no engine FIFO head-of-line-blocks another

PE order: b0.G1 b0.G2 | b1.G1(+b0 softmax) | b0.G4 b0.G3[0:5] | b1.G2 |
          b0.G3[5:8]+b1.G4 (+b1 softmax woven 2/group) | b1.G3
Main GEMMs rotate PSUM banks 0-5; transposes/warm-up own banks 6-7.
Measured: 279442 / 285430 ns on two runs (baseline 326967 ns), err 2.1e-3.
"""

import numpy as np
import ml_dtypes
from contextlib import ExitStack

import concourse.bass as bass
import concourse.tile as tile
from concourse import bacc, mybir, bass_isa
from concourse.bass_utils import run_bass_kernel_spmd

f32 = mybir.dt.float32
f32r = mybir.dt.float32r
bf16 = mybir.dt.bfloat16

P = 128
N = 1024
NT = N // P
NB = 2
NCORES = 8
HALF = 512
BIG = NT * N
Exp = mybir.ActivationFunctionType.Exp
Copy = mybir.ActivationFunctionType.Copy


def _build():
    nc = bacc.Bacc("TRN2", target_bir_lowering=False, debug=False, num_devices=NCORES)

    f1t_d = nc.dram_tensor("f1t", [NB, N, N], f32r, kind="ExternalInput").ap()
    f2t_d = nc.dram_tensor("f2t", [NB, N, N], f32r, kind="ExternalInput").ap()
    f1b_d = nc.dram_tensor("f1b", [NB, N, N], bf16, kind="ExternalInput").ap()
    f2b_d = nc.dram_tensor("f2b", [NB, N, N], bf16, kind="ExternalInput").ap()
    w_d = nc.dram_tensor("w", [N, N], f32r, kind="ExternalInput").ap()
    identb_d = nc.dram_tensor("identb", [P, P], bf16, kind="ExternalInput").ap()
    o1_d = nc.dram_tensor("o1", [NB, N, N], f32, kind="ExternalOutput").ap()
    o2_d = nc.dram_tensor("o2", [NB, N, N], f32, kind="ExternalOutput").ap()

    with tile.TileContext(nc) as tc, ExitStack() as ctx:
        kp = ctx.enter_context(tc.tile_pool(name="kp", bufs=1))
        bigp = ctx.enter_context(tc.tile_pool(name="bigp", bufs=1))
        ccp = ctx.enter_context(tc.tile_pool(name="ccp", bufs=3))
        e2p = ctx.enter_context(tc.tile_pool(name="e2p", bufs=3))
        ostp = ctx.enter_context(tc.tile_pool(name="ostp", bufs=2))
        ost2p = ctx.enter_context(tc.tile_pool(name="ost2p", bufs=2))
        statp = ctx.enter_context(tc.tile_pool(name="statp", bufs=1))
        stat2p = ctx.enter_context(tc.tile_pool(name="stat2p", bufs=2))
        psp = ctx.enter_context(tc.tile_pool(name="psp", bufs=1, space="PSUM"))
        dscrp = ctx.enter_context(tc.tile_pool(name="dscrp", bufs=2, space="DRAM"))

        def ktiles(pfx, tag_pfx, dt, width=N):
            return [kp.tile([P, width], dt, name=f"{pfx}{k}", tag=f"{tag_pfx}{k}")
                    for k in range(NT)]

        wts = ktiles("w", "w", f32r)
        f1s_0 = ktiles("f1_0", "f1", f32r)
        identb = kp.tile([P, P], bf16, name="identb", tag="identb")
        nc.sync.dma_start(identb[:], identb_d[:, :])
        for k in range(NT):
            nc.sync.dma_start(wts[k][:], w_d[k * P:(k + 1) * P, :])
            nc.sync.dma_start(f1s_0[k][:], f1t_d[0, k * P:(k + 1) * P, :])

        # HAM warm-up: ~6us of dummy matmuls on the identity tile while the
        # first input tiles stream in. Without this the clock gate stays at
        # K=4/8 (1.2GHz) deep into G1 because the load-paced matmuls are too
        # sparse to trip the activity monitor.
        ps_warm = psp.tile([P, P], f32, name="ps_warm", tag="ps6")
        for i in range(64):
            nc.tensor.matmul(ps_warm[:], identb[:], identb[:],
                             start=(i == 0), stop=(i == 63))

        a1s = ktiles("a1", "a1", f32r)
        per_b = []
        for b in range(NB):
            d = {}
            d["ccsp"] = dscrp.tile([P, BIG], f32, name=f"ccsp{b}", tag="ccsp")
            d["scr"] = dscrp.tile([1, N], f32, name=f"scr{b}", tag="scr")
            d["nvmax"] = stat2p.tile([P, NT], f32, name=f"nvmax{b}", tag="nvmax")
            d["vs"] = stat2p.tile([P, NT], f32, name=f"vs{b}", tag="vs")
            d["rv"] = stat2p.tile([P, NT], f32, name=f"rv{b}", tag="rv")
            d["rsa"] = stat2p.tile([P, NT], f32, name=f"rsa{b}", tag="rsa")
            per_b.append(d)

        # ---- GEMM group: k-inner, both 512-halves share each LDWEIGHTS ----
        # drain halves go to DVE (n=0) and ACT (n=1) in parallel.
        def gemm(lhsT_sl, rhs_sl, m, drain0, drain1, pstag, psbase=None):
            pst = (m % 3) * 2 if psbase is None else psbase
            ps0 = psp.tile([P, HALF], f32, name=f"ps_{pstag}_{m}_0",
                           tag=f"ps{pst}")
            ps1 = psp.tile([P, HALF], f32, name=f"ps_{pstag}_{m}_1",
                           tag=f"ps{pst + 1}")
            for k in range(NT):
                nc.tensor.matmul(ps0[:], lhsT_sl(k, m), rhs_sl(k, 0),
                                 start=(k == 0), stop=(k == NT - 1))
                nc.tensor.matmul(ps1[:], lhsT_sl(k, m), rhs_sl(k, 1),
                                 start=(k == 0), stop=(k == NT - 1))
            drain0(m, 0, ps0)
            drain1(m, 1, ps1)

        def sl_k(tiles):
            return lambda k, m: tiles[k][:, m * P:(m + 1) * P]

        def sl_kr(tiles, base=0):
            return lambda k, n: tiles[k][:, base + n * HALF: base + (n + 1) * HALF]

        def sl_big(t):
            return lambda k, m: t[:, k * N + m * P: k * N + (m + 1) * P]

        state = {}

        # ---------------- G1: k-outer passes, paces with per-k loads -------
        # passes of 3/3/2 m-tiles use only PSUM banks 0-5, leaving 6/7 to
        # the e2 transposes that run concurrently during G2 phases
        def g1(b, f1_tiles, after_pass0=None):
            for mlo, mhi in ((0, 3), (3, 6), (6, 8)):
                if mlo == 3 and after_pass0 is not None:
                    after_pass0()
                pss = []
                for mi, m in enumerate(range(mlo, mhi)):
                    pss.append((
                        psp.tile([P, HALF], f32, name=f"psg1_{b}_{m}_0",
                                 tag=f"ps{mi * 2}"),
                        psp.tile([P, HALF], f32, name=f"psg1_{b}_{m}_1",
                                 tag=f"ps{mi * 2 + 1}")))
                for k in range(NT):
                    for mi, m in enumerate(range(mlo, mhi)):
                        lh = wts[k][:, m * P:(m + 1) * P]
                        nc.tensor.matmul(pss[mi][0][:], lh,
                                         f1_tiles[k][:, 0:HALF],
                                         start=(k == 0), stop=(k == NT - 1))
                        nc.tensor.matmul(pss[mi][1][:], lh,
                                         f1_tiles[k][:, HALF:N],
                                         start=(k == 0), stop=(k == NT - 1))
                for mi, m in enumerate(range(mlo, mhi)):
                    nc.vector.tensor_copy(a1s[m][:, 0:HALF], pss[mi][0][:])
                    nc.scalar.copy(a1s[m][:, HALF:N], pss[mi][1][:])

        # ---------------- G2 + per-m softmax-2 (e2) path -------------------
        # e2T is built by PE transpose-mode matmuls (bf16, 1 cyc/row) into
        # the reserved PSUM banks 6/7, lagged one m-tile behind the GEMM so
        # the PE never waits on the exp's cross-engine latency. g2 returns a
        # closure emitting the last tile's transposes, which the caller
        # places after the next phase's first PE group.
        def transpose_tile(b, m, e2t, e2t_t):
            e2tv = e2t_t[:, :].rearrange("p (j c) -> p j c", c=N)
            for j0 in range(2):
                psT = psp.tile([P, HALF], bf16, name=f"psT_{b}_{m}_{j0}",
                               tag=f"ps{6 + j0}")
                for q in range(4):
                    j = 4 * j0 + q
                    nc.tensor.matmul(psT[:, q * P:(q + 1) * P],
                                     e2t[:, j * P:(j + 1) * P], identb[:],
                                     is_transpose=True,
                                     start=(q == 0), stop=(q == 3))
                dr = nc.vector.tensor_copy if j0 == 0 else nc.scalar.copy
                dr(e2tv[:, 4 * j0:4 * j0 + 4, m * P:(m + 1) * P],
                   psT[:, :].rearrange("p (j c) -> p j c", c=P))

        def g2(b, f2_tiles, e2t_t):
            d = per_b[b]
            amaxacc = statp.tile([P, N], f32, name=f"amaxacc{b}", tag="amaxacc")
            ccs = []
            e2s = []

            def drain0(m, n, ps):
                nc.vector.tensor_copy(ccs[m][:, 0:HALF], ps[:])

            def drain1(m, n, ps):
                nc.scalar.copy(ccs[m][:, HALF:N], ps[:])

            for m in range(NT):
                cct = ccp.tile([P, N], f32, name=f"cc_{b}_{m}", tag="cc")
                ccs.append(cct)
                gemm(sl_k(a1s), sl_kr(f2_tiles), m, drain0, drain1, f"cc_{b}")
                if m > 0:
                    transpose_tile(b, m - 1, e2s[m - 1], e2t_t)
                nc.scalar.dma_start(d["ccsp"][:, m * N:(m + 1) * N], cct[:])
                nc.vector.tensor_reduce(
                    out=d["nvmax"][:, m:m + 1], in_=cct[:],
                    axis=mybir.AxisListType.X, op=mybir.AluOpType.max,
                    negate=True)
                e2t = e2p.tile([P, N], bf16, name=f"e2_{b}_{m}", tag="e2")
                e2s.append(e2t)
                # exp with per-partition bias; accum_out = row sum = vsum
                nc.scalar.activation(e2t[:], cct[:], Exp,
                                     bias=d["nvmax"][:, m:m + 1],
                                     accum_out=d["vs"][:, m:m + 1])
                if m == 0:
                    nc.vector.tensor_copy(amaxacc[:], cct[:])
                else:
                    nc.vector.tensor_tensor(
                        out=amaxacc[:], in0=amaxacc[:], in1=cct[:],
                        op=mybir.AluOpType.max)
            nc.vector.reciprocal(d["rv"][:], d["vs"][:])
            state[b] = dict(amaxacc=amaxacc)
            return lambda: transpose_tile(b, NT - 1, e2s[NT - 1], e2t_t)

        # ---------------- softmax-1 (e1) path, emitted piecewise -----------
        def sm1_start(b):
            d = per_b[b]
            amaxB = statp.tile([P, N], f32, name=f"amaxB{b}", tag="amaxB")
            nc.gpsimd.partition_all_reduce(
                amaxB[:], state[b]["amaxacc"][:], channels=P,
                reduce_op=bass_isa.ReduceOp.max)
            asumacc = statp.tile([P, N], f32, name=f"asumacc{b}", tag="asumacc")
            state[b].update(amaxB=amaxB, asumacc=asumacc)

        def _sm1_add(b, m, combo_tiles):
            st = state[b]
            if m == 0:
                nc.vector.tensor_copy(st["asumacc"][:], combo_tiles[m][:, 0:N])
            else:
                nc.vector.tensor_tensor(
                    out=st["asumacc"][:], in0=st["asumacc"][:],
                    in1=combo_tiles[m][:, 0:N], op=mybir.AluOpType.add)

        def sm1_piece(b, m, combo_tiles):
            """reload+sub+exp for tile m; the asum add lags one tile so the
            DVE never queue-blocks on this tile's ACT exp."""
            d = per_b[b]
            st = state[b]
            ccr = ccp.tile([P, N], f32, name=f"ccr_{b}_{m}", tag="cc")
            nc.sync.dma_start(ccr[:], d["ccsp"][:, m * N:(m + 1) * N])
            nc.vector.tensor_tensor(
                out=ccr[:], in0=ccr[:], in1=st["amaxB"][:],
                op=mybir.AluOpType.subtract)
            nc.scalar.activation(combo_tiles[m][:, 0:N], ccr[:], Exp)
            if m > 0:
                _sm1_add(b, m - 1, combo_tiles)

        def sm1_finish(b, combo_tiles):
            d = per_b[b]
            st = state[b]
            _sm1_add(b, NT - 1, combo_tiles)
            asumB = statp.tile([P, N], f32, name=f"asumB{b}", tag="amaxB")
            nc.gpsimd.partition_all_reduce(
                asumB[:], st["asumacc"][:], channels=P,
                reduce_op=bass_isa.ReduceOp.add)
            nc.sync.dma_start(d["scr"][0:1, :], asumB[0:1, :])
            nc.sync.dma_start(
                d["rsa"][:],
                d["scr"][0:1, :].rearrange("one (m p) -> (one p) m", p=P))
            nc.vector.reciprocal(d["rsa"][:], d["rsa"][:])

        # ---------------- output GEMMs -------------------------------------
        def g4_group(b, e2t_t, f2b_tiles, m):
            d = per_b[b]
            ost = ostp.tile([P, N], f32, name=f"ost4_{b}", tag="ost")

            def drain0(m_, n, ps):
                nc.vector.tensor_scalar_mul(
                    ost[:, 0:HALF], ps[:], d["rv"][:, m_:m_ + 1])

            def drain1(m_, n, ps):
                nc.scalar.activation(ost[:, HALF:N], ps[:], Copy,
                                     bias=0.0, scale=d["rv"][:, m_:m_ + 1])
            gemm(sl_big(e2t_t), sl_kr(f2b_tiles), m, drain0, drain1,
                 f"r2_{b}")
            nc.scalar.dma_start(o2_d[b, m * P:(m + 1) * P, :], ost[:])

        def g3_group(b, combo_tiles, m, psbase=None, last=False):
            d = per_b[b]
            ost = ost2p.tile([P, N], f32, name=f"ost3_{b}", tag="ost2")

            def drain0(m_, n, ps):
                nc.vector.tensor_scalar_mul(
                    ost[:, 0:HALF], ps[:], d["rsa"][:, m_:m_ + 1])

            def drain1(m_, n, ps):
                nc.scalar.activation(ost[:, HALF:N], ps[:], Copy,
                                     bias=0.0, scale=d["rsa"][:, m_:m_ + 1])
            gemm(sl_k(combo_tiles), sl_kr(combo_tiles, base=N), m,
                 drain0, drain1, f"r1_{b}", psbase=psbase)
            if last:
                # split the final store across both hwdge queues so the
                # end-of-kernel barrier waits half the transfer
                nc.sync.dma_start(
                    o1_d[b, m * P:(m + 1) * P, 0:HALF], ost[:, 0:HALF])
                nc.scalar.dma_start(
                    o1_d[b, m * P:(m + 1) * P, HALF:N], ost[:, HALF:N])
            else:
                nc.scalar.dma_start(o1_d[b, m * P:(m + 1) * P, :], ost[:])

        # ================= global schedule =================================
        # dummy gpsimd custom op: forces the Pool LOAD_LIB during idle P1
        # instead of on the first latency-critical all-reduce
        dummy = stat2p.tile([P, 1], f32, name="dummy", tag="dummy")
        nc.vector.memset(dummy[:], 0.0)
        nc.gpsimd.partition_all_reduce(dummy[:], dummy[:], channels=P,
                                       reduce_op=bass_isa.ReduceOp.max)

        f2s_0 = ktiles("f2_0", "f2", f32r)
        f2bs_0 = ktiles("f2b_0", "f2b", bf16)
        for k in range(NT):
            nc.sync.dma_start(f2s_0[k][:], f2t_d[0, k * P:(k + 1) * P, :])
            nc.sync.dma_start(f2bs_0[k][:], f2b_d[0, k * P:(k + 1) * P, :])

        # P1: b0.G1
        g1(0, f1s_0)

        f1s_1 = [kp.tile([P, N], f32r, name=f"f1_1{k}", tag=f"f1{k}")
                 for k in range(NT)]
        for k in range(NT):
            nc.sync.dma_start(f1s_1[k][:], f1t_d[1, k * P:(k + 1) * P, :])

        # P2: b0.G2
        e2t_0 = bigp.tile([P, BIG], bf16, name="e2t_0", tag="e2t")
        tr_last_0 = g2(0, f2s_0, e2t_0)

        combo_0 = [kp.tile([P, 2 * N], bf16, name=f"combo_0{k}", tag=f"f2{k}")
                   for k in range(NT)]

        # P3: b1.G1 with b0's softmax-1 pieces around it (subs before the
        # G1 drains hit the DVE FIFO, rest after). The m7 transposes of b0
        # slot in behind b1.G1's first MMs so they never stall the PE on
        # the exp latency.
        sm1_start(0)
        for m in range(0, 3):
            sm1_piece(0, m, combo_0)
        g1(1, f1s_1, after_pass0=tr_last_0)
        for m in range(3, NT):
            sm1_piece(0, m, combo_0)
        # f1b half of combo_0 loads after the e1 exps (same-tile writers);
        # gpsimd queue so the sync load stream stays clean
        for k in range(NT):
            nc.gpsimd.dma_start(combo_0[k][:, N:2 * N],
                                f1b_d[0, k * P:(k + 1) * P, :])

        f2s_1 = [kp.tile([P, N], f32r, name=f"f2_1{k}", tag=f"f1{k}")
                 for k in range(NT)]
        for k in range(NT):
            nc.sync.dma_start(f2s_1[k][:], f2t_d[1, k * P:(k + 1) * P, :])
        f2bs_1 = ktiles("f2b_1", "f2b", bf16)
        for k in range(NT):
            nc.sync.dma_start(f2bs_1[k][:], f2b_d[1, k * P:(k + 1) * P, :])

        # P4: b0.G4; sm1_finish after it so the rsa reciprocal sits behind
        # the G4 drains in the DVE FIFO (rsa is only needed at P5)
        for m in range(NT):
            g4_group(0, e2t_0, f2bs_0, m)
        sm1_finish(0, combo_0)

        # combo_1's f1b half loads now: its WAR (f1_1 readers) cleared at
        # P3-end, and it must not queue behind b1's amax all-reduce on
        # gpsimd (which can only run after P6)
        combo_1 = [kp.tile([P, 2 * N], bf16, name=f"combo_1{k}", tag=f"w{k}")
                   for k in range(NT)]
        for k in range(NT):
            nc.gpsimd.dma_start(combo_1[k][:, N:2 * N],
                                f1b_d[1, k * P:(k + 1) * P, :])

        # P5: b0.G3 first 5 m-tiles
        for m in range(0, 5):
            g3_group(0, combo_0, m)

        # P6: b1.G2
        e2t_1 = bigp.tile([P, BIG], bf16, name="e2t_1", tag="e2t")
        tr_last_1 = g2(1, f2s_1, e2t_1)

        # P7: b0.G3 tail + b1.G4, with b1's softmax-1 woven per-group;
        # b1's m7 transposes slot in behind the first tail group
        sm1_start(1)
        weave = iter(range(NT))
        first_tail = True
        for m in range(5, NT):
            g3_group(0, combo_0, m)
            if first_tail:
                tr_last_1()
                first_tail = False
            for wm in (next(weave, None), next(weave, None)):
                if wm is not None:
                    sm1_piece(1, wm, combo_1)
        for m in range(NT):
            g4_group(1, e2t_1, f2bs_1, m)
            wm = next(weave, None)
            if wm is not None:
                sm1_piece(1, wm, combo_1)
        sm1_finish(1, combo_1)

        # P8: b1.G3 — rotate PSUM banks [6,4,0,2]: banks 6/7 are free after
        # b1's transposes, so the first group never WAR-waits on b1.G4's
        # trailing drains, and the 4-deep cycle gives extra drain grace
        for m in range(NT):
            g3_group(1, combo_1, m, psbase=(6, 4, 0, 2)[m % 4],
                     last=(m == NT - 1))

    nc.compile()
    return nc


_NC = None
TRACE = False
LAST = None


def _get_nc():
    global _NC
    if _NC is None:
        _NC = _build()
    return _NC


def kernel(f1_norm, f2_norm, corr_weights):
    f1_norm = np.ascontiguousarray(f1_norm, dtype=np.float32)
    f2_norm = np.ascontiguousarray(f2_norm, dtype=np.float32)
    w = np.ascontiguousarray(corr_weights, dtype=np.float32)
    B = f1_norm.shape[0]
    assert B == NB * NCORES

    f1t = np.ascontiguousarray(np.swapaxes(f1_norm, 1, 2))
    f2t = np.ascontiguousarray(np.swapaxes(f2_norm, 1, 2))
    f1b = f1t.astype(ml_dtypes.bfloat16)
    f2b = f2t.astype(ml_dtypes.bfloat16)
    identb = np.eye(P, dtype=ml_dtypes.bfloat16)

    nc = _get_nc()
    in_maps = [
        {"f1t": f1t[c * NB:(c + 1) * NB], "f2t": f2t[c * NB:(c + 1) * NB],
         "f1b": f1b[c * NB:(c + 1) * NB], "f2b": f2b[c * NB:(c + 1) * NB],
         "w": w, "identb": identb}
        for c in range(NCORES)
    ]
    res = run_bass_kernel_spmd(nc, in_maps, core_ids=list(range(NCORES)), trace=TRACE)
    global LAST
    LAST = res
    out1 = np.concatenate([res.results[c]["o1"] for c in range(NCORES)], axis=0)
    out2 = np.concatenate([res.results[c]["o2"] for c in range(NCORES)], axis=0)
    return out1, out2


# revision 66
# speedup vs baseline: 1.0554x; 1.0043x over previous
"""Trainium2 Bass kernel for nn_CAM_41377714929724 (CAM cross-attention module).

  a1  = f1 @ W                      [B,S,D]
  cc  = a1 @ f2^T                   [B,S,S]
  aatt = softmax(cc, axis=s)        (over rows -> column-normalized)
  vatt = softmax(cc, axis=t).T      (over cols, transposed)
  out1 = (f1 @ aatt).swap(1,2)      [B,S,S]
  out2 = (f2 @ vatt).swap(1,2)      [B,S,S]

Sharding: pure data parallelism, 2 batches per core on 8 cores; W replicated.

PE runs only the four 1024^3 GEMMs per batch (G1/G2 f32r for the precision-
sensitive logit chain, G3/G4 bf16 post-softmax) plus cheap bf16 transpose-
mode matmuls and a ~6us HAM warm-up burst. Everything else lives on
DVE/ACT/gpsimd/DMA:
 - G1 runs k-outer in passes of 3/3/2 m-tiles so its matmuls pace with the
   per-k-tile input DMAs; each f32r GEMM group runs its two 512-wide halves
   k-inner so both share every (forced, self-loaded) LDWEIGHTS
 - e2 = exp(cc - rowmax) via ACT with per-partition bias; the same ACT op
   emits vsum through accum_out (free-dim sum) -> no separate reduce
 - e2T via PE transpose-mode (bf16, PSUM banks 6/7), lagged one m-tile so
   the PE never waits on the exp's cross-engine latency (the DMA xbar
   alternative serializes at ~3.5us per [128,512] call - far too slow)
 - cc spilled to DRAM (scalar queue), reloaded (sync) for e1 = exp(cc -
   colmax); colmax via DVE running max + gpsimd partition_all_reduce
   (library preloaded by a dummy op at t=0)
 - asum -> 1/asum scale vector via DRAM bounce (tiny [128,8] reciprocal)
 - every GEMM group drains its two 512-halves on DVE and ACT in parallel;
   queue discipline keeps sync = loads/reloads/bounces, scalar = spills/
   stores/drains, gpsimd = late bf16 loads + all-reduces
 - per-batch softmax work is woven instruction-by-instruction into the other
   batch's PE filler loops so no engine FIFO head-of-line-blocks another

PE order: b0.G1 b0.G2 | b1.G1(+b0 softmax) | b0.G4 b0.G3[0:5] | b1.G2 |
          b0.G3[5:8]+b1.G4 (+b1 softmax woven 2/group) | b1.G3
Main GEMMs rotate PSUM banks 0-5; transposes/warm-up own banks 6-7.
P8 rotates PSUM banks [6,4,0,2] (6/7 free post-transpose) and the final
store splits across both hwdge queues to shorten the end barrier.
Measured: 280762 ns (prior best 279442/285430; baseline 326967), err 2.1e-3.
"""

import numpy as np
import ml_dtypes
from contextlib import ExitStack

import concourse.bass as bass
import concourse.tile as tile
from concourse import bacc, mybir, bass_isa
from concourse.bass_utils import run_bass_kernel_spmd

f32 = mybir.dt.float32
f32r = mybir.dt.float32r
bf16 = mybir.dt.bfloat16

P = 128
N = 1024
NT = N // P
NB = 2
NCORES = 8
HALF = 512
BIG = NT * N
Exp = mybir.ActivationFunctionType.Exp
Copy = mybir.ActivationFunctionType.Copy


def _build():
    nc = bacc.Bacc("TRN2", target_bir_lowering=False, debug=False, num_devices=NCORES)

    f1t_d = nc.dram_tensor("f1t", [NB, N, N], f32r, kind="ExternalInput").ap()
    f2t_d = nc.dram_tensor("f2t", [NB, N, N], f32r, kind="ExternalInput").ap()
    f1b_d = nc.dram_tensor("f1b", [NB, N, N], bf16, kind="ExternalInput").ap()
    f2b_d = nc.dram_tensor("f2b", [NB, N, N], bf16, kind="ExternalInput").ap()
    w_d = nc.dram_tensor("w", [N, N], f32r, kind="ExternalInput").ap()
    identb_d = nc.dram_tensor("identb", [P, P], bf16, kind="ExternalInput").ap()
    o1_d = nc.dram_tensor("o1", [NB, N, N], f32, kind="ExternalOutput").ap()
    o2_d = nc.dram_tensor("o2", [NB, N, N], f32, kind="ExternalOutput").ap()

    with tile.TileContext(nc) as tc, ExitStack() as ctx:
        kp = ctx.enter_context(tc.tile_pool(name="kp", bufs=1))
        bigp = ctx.enter_context(tc.tile_pool(name="bigp", bufs=1))
        ccp = ctx.enter_context(tc.tile_pool(name="ccp", bufs=3))
        e2p = ctx.enter_context(tc.tile_pool(name="e2p", bufs=3))
        ostp = ctx.enter_context(tc.tile_pool(name="ostp", bufs=2))
        ost2p = ctx.enter_context(tc.tile_pool(name="ost2p", bufs=2))
        statp = ctx.enter_context(tc.tile_pool(name="statp", bufs=1))
        stat2p = ctx.enter_context(tc.tile_pool(name="stat2p", bufs=2))
        psp = ctx.enter_context(tc.tile_pool(name="psp", bufs=1, space="PSUM"))
        dscrp = ctx.enter_context(tc.tile_pool(name="dscrp", bufs=2, space="DRAM"))

        def ktiles(pfx, tag_pfx, dt, width=N):
            return [kp.tile([P, width], dt, name=f"{pfx}{k}", tag=f"{tag_pfx}{k}")
                    for k in range(NT)]

        wts = ktiles("w", "w", f32r)
        f1s_0 = ktiles("f1_0", "f1", f32r)
        identb = kp.tile([P, P], bf16, name="identb", tag="identb")
        nc.sync.dma_start(identb[:], identb_d[:, :])
        for k in range(NT):
            nc.sync.dma_start(wts[k][:], w_d[k * P:(k + 1) * P, :])
            nc.sync.dma_start(f1s_0[k][:], f1t_d[0, k * P:(k + 1) * P, :])

        # HAM warm-up: ~6us of dummy matmuls on the identity tile while the
        # first input tiles stream in. Without this the clock gate stays at
        # K=4/8 (1.2GHz) deep into G1 because the load-paced matmuls are too
        # sparse to trip the activity monitor.
        ps_warm = psp.tile([P, P], f32, name="ps_warm", tag="ps6")
        for i in range(64):
            nc.tensor.matmul(ps_warm[:], identb[:], identb[:],
                             start=(i == 0), stop=(i == 63))

        a1s = ktiles("a1", "a1", f32r)
        per_b = []
        for b in range(NB):
            d = {}
            d["ccsp"] = dscrp.tile([P, BIG], f32, name=f"ccsp{b}", tag="ccsp")
            d["scr"] = dscrp.tile([1, N], f32, name=f"scr{b}", tag="scr")
            d["nvmax"] = stat2p.tile([P, NT], f32, name=f"nvmax{b}", tag="nvmax")
            d["vs"] = stat2p.tile([P, NT], f32, name=f"vs{b}", tag="vs")
            d["rv"] = stat2p.tile([P, NT], f32, name=f"rv{b}", tag="rv")
            d["rsa"] = stat2p.tile([P, NT], f32, name=f"rsa{b}", tag="rsa")
            per_b.append(d)

        # ---- GEMM group: k-inner, both 512-halves share each LDWEIGHTS ----
        # drain halves go to DVE (n=0) and ACT (n=1) in parallel.
        def gemm(lhsT_sl, rhs_sl, m, drain0, drain1, pstag, psbase=None):
            pst = (m % 3) * 2 if psbase is None else psbase
            ps0 = psp.tile([P, HALF], f32, name=f"ps_{pstag}_{m}_0",
                           tag=f"ps{pst}")
            ps1 = psp.tile([P, HALF], f32, name=f"ps_{pstag}_{m}_1",
                           tag=f"ps{pst + 1}")
            for k in range(NT):
                nc.tensor.matmul(ps0[:], lhsT_sl(k, m), rhs_sl(k, 0),
                                 start=(k == 0), stop=(k == NT - 1))
                nc.tensor.matmul(ps1[:], lhsT_sl(k, m), rhs_sl(k, 1),
                                 start=(k == 0), stop=(k == NT - 1))
            drain0(m, 0, ps0)
            drain1(m, 1, ps1)

        def sl_k(tiles):
            return lambda k, m: tiles[k][:, m * P:(m + 1) * P]

        def sl_kr(tiles, base=0):
            return lambda k, n: tiles[k][:, base + n * HALF: base + (n + 1) * HALF]

        def sl_big(t):
            return lambda k, m: t[:, k * N + m * P: k * N + (m + 1) * P]

        state = {}

        # ---------------- G1: k-outer passes, paces with per-k loads -------
        # passes of 3/3/2 m-tiles use only PSUM banks 0-5, leaving 6/7 to
        # the e2 transposes that run concurrently during G2 phases
        def g1(b, f1_tiles, after_pass0=None):
            for mlo, mhi in ((0, 3), (3, 6), (6, 8)):
                if mlo == 3 and after_pass0 is not None:
                    after_pass0()
                pss = []
                for mi, m in enumerate(range(mlo, mhi)):
                    pss.append((
                        psp.tile([P, HALF], f32, name=f"psg1_{b}_{m}_0",
                                 tag=f"ps{mi * 2}"),
                        psp.tile([P, HALF], f32, name=f"psg1_{b}_{m}_1",
                                 tag=f"ps{mi * 2 + 1}")))
                for k in range(NT):
                    for mi, m in enumerate(range(mlo, mhi)):
                        lh = wts[k][:, m * P:(m + 1) * P]
                        nc.tensor.matmul(pss[mi][0][:], lh,
                                         f1_tiles[k][:, 0:HALF],
                                         start=(k == 0), stop=(k == NT - 1))
                        nc.tensor.matmul(pss[mi][1][:], lh,
                                         f1_tiles[k][:, HALF:N],
                                         start=(k == 0), stop=(k == NT - 1))
                for mi, m in enumerate(range(mlo, mhi)):
                    nc.vector.tensor_copy(a1s[m][:, 0:HALF], pss[mi][0][:])
                    nc.scalar.copy(a1s[m][:, HALF:N], pss[mi][1][:])

        # ---------------- G2 + per-m softmax-2 (e2) path -------------------
        # e2T is built by PE transpose-mode matmuls (bf16, 1 cyc/row) into
        # the reserved PSUM banks 6/7, lagged one m-tile behind the GEMM so
        # the PE never waits on the exp's cross-engine latency. g2 returns a
        # closure emitting the last tile's transposes, which the caller
        # places after the next phase's first PE group.
        def transpose_tile(b, m, e2t, e2t_t):
            e2tv = e2t_t[:, :].rearrange("p (j c) -> p j c", c=N)
            for j0 in range(2):
                psT = psp.tile([P, HALF], bf16, name=f"psT_{b}_{m}_{j0}",
                               tag=f"ps{6 + j0}")
                for q in range(4):
                    j = 4 * j0 + q
                    nc.tensor.matmul(psT[:, q * P:(q + 1) * P],
                                     e2t[:, j * P:(j + 1) * P], identb[:],
                                     is_transpose=True,
                                     start=(q == 0), stop=(q == 3))
                dr = nc.vector.tensor_copy if j0 == 0 else nc.scalar.copy
                dr(e2tv[:, 4 * j0:4 * j0 + 4, m * P:(m + 1) * P],
                   psT[:, :].rearrange("p (j c) -> p j c", c=P))

        def g2(b, f2_tiles, e2t_t):
            d = per_b[b]
            amaxacc = statp.tile([P, N], f32, name=f"amaxacc{b}", tag="amaxacc")
            ccs = []
            e2s = []

            def drain0(m, n, ps):
                nc.vector.tensor_copy(ccs[m][:, 0:HALF], ps[:])

            def drain1(m, n, ps):
                nc.scalar.copy(ccs[m][:, HALF:N], ps[:])

            for m in range(NT):
                cct = ccp.tile([P, N], f32, name=f"cc_{b}_{m}", tag="cc")
                ccs.append(cct)
                gemm(sl_k(a1s), sl_kr(f2_tiles), m, drain0, drain1, f"cc_{b}")
                if m > 0:
                    transpose_tile(b, m - 1, e2s[m - 1], e2t_t)
                nc.scalar.dma_start(d["ccsp"][:, m * N:(m + 1) * N], cct[:])
                nc.vector.tensor_reduce(
                    out=d["nvmax"][:, m:m + 1], in_=cct[:],
                    axis=mybir.AxisListType.X, op=mybir.AluOpType.max,
                    negate=True)
                e2t = e2p.tile([P, N], bf16, name=f"e2_{b}_{m}", tag="e2")
                e2s.append(e2t)
                # exp with per-partition bias; accum_out = row sum = vsum
                nc.scalar.activation(e2t[:], cct[:], Exp,
                                     bias=d["nvmax"][:, m:m + 1],
                                     accum_out=d["vs"][:, m:m + 1])
                if m == 0:
                    nc.vector.tensor_copy(amaxacc[:], cct[:])
                else:
                    nc.vector.tensor_tensor(
                        out=amaxacc[:], in0=amaxacc[:], in1=cct[:],
                        op=mybir.AluOpType.max)
            nc.vector.reciprocal(d["rv"][:], d["vs"][:])
            state[b] = dict(amaxacc=amaxacc)
            return lambda: transpose_tile(b, NT - 1, e2s[NT - 1], e2t_t)

        # ---------------- softmax-1 (e1) path, emitted piecewise -----------
        def sm1_start(b):
            d = per_b[b]
            amaxB = statp.tile([P, N], f32, name=f"amaxB{b}", tag="amaxB")
            nc.gpsimd.partition_all_reduce(
                amaxB[:], state[b]["amaxacc"][:], channels=P,
                reduce_op=bass_isa.ReduceOp.max)
            asumacc = statp.tile([P, N], f32, name=f"asumacc{b}", tag="asumacc")
            state[b].update(amaxB=amaxB, asumacc=asumacc)

        def _sm1_add(b, m, combo_tiles):
            st = state[b]
            if m == 0:
                nc.vector.tensor_copy(st["asumacc"][:], combo_tiles[m][:, 0:N])
            else:
                nc.vector.tensor_tensor(
                    out=st["asumacc"][:], in0=st["asumacc"][:],
                    in1=combo_tiles[m][:, 0:N], op=mybir.AluOpType.add)

        def sm1_piece(b, m, combo_tiles):
            """reload+sub+exp for tile m; the asum add lags one tile so the
            DVE never queue-blocks on this tile's ACT exp."""
            d = per_b[b]
            st = state[b]
            ccr = ccp.tile([P, N], f32, name=f"ccr_{b}_{m}", tag="cc")
            nc.sync.dma_start(ccr[:], d["ccsp"][:, m * N:(m + 1) * N])
            nc.vector.tensor_tensor(
                out=ccr[:], in0=ccr[:], in1=st["amaxB"][:],
                op=mybir.AluOpType.subtract)
            nc.scalar.activation(combo_tiles[m][:, 0:N], ccr[:], Exp)
            if m > 0:
                _sm1_add(b, m - 1, combo_tiles)

        def sm1_finish(b, combo_tiles):
            d = per_b[b]
            st = state[b]
            _sm1_add(b, NT - 1, combo_tiles)
            asumB = statp.tile([P, N], f32, name=f"asumB{b}", tag="amaxB")
            nc.gpsimd.partition_all_reduce(
                asumB[:], st["asumacc"][:], channels=P,
                reduce_op=bass_isa.ReduceOp.add)
            nc.sync.dma_start(d["scr"][0:1, :], asumB[0:1, :])
            nc.sync.dma_start(
                d["rsa"][:],
                d["scr"][0:1, :].rearrange("one (m p) -> (one p) m", p=P))
            nc.vector.reciprocal(d["rsa"][:], d["rsa"][:])

        # ---------------- output GEMMs -------------------------------------
        def g4_group(b, e2t_t, f2b_tiles, m, psbase=None):
            d = per_b[b]
            ost = ostp.tile([P, N], f32, name=f"ost4_{b}", tag="ost")

            def drain0(m_, n, ps):
                nc.vector.tensor_scalar_mul(
                    ost[:, 0:HALF], ps[:], d["rv"][:, m_:m_ + 1])

            def drain1(m_, n, ps):
                nc.scalar.activation(ost[:, HALF:N], ps[:], Copy,
                                     bias=0.0, scale=d["rv"][:, m_:m_ + 1])
            gemm(sl_big(e2t_t), sl_kr(f2b_tiles), m, drain0, drain1,
                 f"r2_{b}", psbase=psbase)
            nc.scalar.dma_start(o2_d[b, m * P:(m + 1) * P, :], ost[:])

        def g3_group(b, combo_tiles, m, psbase=None, last=False):
            d = per_b[b]
            ost = ost2p.tile([P, N], f32, name=f"ost3_{b}", tag="ost2")

            def drain0(m_, n, ps):
                nc.vector.tensor_scalar_mul(
                    ost[:, 0:HALF], ps[:], d["rsa"][:, m_:m_ + 1])

            def drain1(m_, n, ps):
                nc.scalar.activation(ost[:, HALF:N], ps[:], Copy,
                                     bias=0.0, scale=d["rsa"][:, m_:m_ + 1])
            gemm(sl_k(combo_tiles), sl_kr(combo_tiles, base=N), m,
                 drain0, drain1, f"r1_{b}", psbase=psbase)
            if last:
                # split the final store across both hwdge queues so the
                # end-of-kernel barrier waits half the transfer
                nc.sync.dma_start(
                    o1_d[b, m * P:(m + 1) * P, 0:HALF], ost[:, 0:HALF])
                nc.scalar.dma_start(
                    o1_d[b, m * P:(m + 1) * P, HALF:N], ost[:, HALF:N])
            else:
                nc.scalar.dma_start(o1_d[b, m * P:(m + 1) * P, :], ost[:])

        # ================= global schedule =================================
        # dummy gpsimd custom op: forces the Pool LOAD_LIB during idle P1
        # instead of on the first latency-critical all-reduce
        dummy = stat2p.tile([P, 1], f32, name="dummy", tag="dummy")
        nc.vector.memset(dummy[:], 0.0)
        nc.gpsimd.partition_all_reduce(dummy[:], dummy[:], channels=P,
                                       reduce_op=bass_isa.ReduceOp.max)

        f2s_0 = ktiles("f2_0", "f2", f32r)
        f2bs_0 = ktiles("f2b_0", "f2b", bf16)
        for k in range(NT):
            nc.sync.dma_start(f2s_0[k][:], f2t_d[0, k * P:(k + 1) * P, :])

        # P1: b0.G1
        g1(0, f1s_0)

        # f1_1 ahead of f2b_0 in the sync queue: b1.G1 (P3) needs f1_1,
        # while f2b_0 is only read at P4 — better SDMA queue positions for
        # the latency-critical tensor
        f1s_1 = [kp.tile([P, N], f32r, name=f"f1_1{k}", tag=f"f1{k}")
                 for k in range(NT)]
        for k in range(NT):
            nc.sync.dma_start(f1s_1[k][:], f1t_d[1, k * P:(k + 1) * P, :])
        for k in range(NT):
            nc.sync.dma_start(f2bs_0[k][:], f2b_d[0, k * P:(k + 1) * P, :])

        # P2: b0.G2
        e2t_0 = bigp.tile([P, BIG], bf16, name="e2t_0", tag="e2t")
        tr_last_0 = g2(0, f2s_0, e2t_0)

        combo_0 = [kp.tile([P, 2 * N], bf16, name=f"combo_0{k}", tag=f"f2{k}")
                   for k in range(NT)]

        # P3: b1.G1 with b0's softmax-1 pieces around it (subs before the
        # G1 drains hit the DVE FIFO, rest after). The m7 transposes of b0
        # slot in behind b1.G1's first MMs so they never stall the PE on
        # the exp latency.
        sm1_start(0)
        for m in range(0, 3):
            sm1_piece(0, m, combo_0)
        g1(1, f1s_1, after_pass0=tr_last_0)
        for m in range(3, NT):
            sm1_piece(0, m, combo_0)
        # f1b half of combo_0 loads after the e1 exps (same-tile writers);
        # gpsimd queue so the sync load stream stays clean
        for k in range(NT):
            nc.gpsimd.dma_start(combo_0[k][:, N:2 * N],
                                f1b_d[0, k * P:(k + 1) * P, :])

        f2s_1 = [kp.tile([P, N], f32r, name=f"f2_1{k}", tag=f"f1{k}")
                 for k in range(NT)]
        for k in range(NT):
            nc.sync.dma_start(f2s_1[k][:], f2t_d[1, k * P:(k + 1) * P, :])
        f2bs_1 = ktiles("f2b_1", "f2b", bf16)
        for k in range(NT):
            nc.sync.dma_start(f2bs_1[k][:], f2b_d[1, k * P:(k + 1) * P, :])

        # P4: b0.G4; sm1_finish after it so the rsa reciprocal sits behind
        # the G4 drains in the DVE FIFO (rsa is only needed at P5).
        # Bank rotation (4,0,2) starts on banks released by b1.G1's middle
        # pass instead of colliding with its final pass (banks 0-3).
        for m in range(NT):
            g4_group(0, e2t_0, f2bs_0, m, psbase=(4, 0, 2)[m % 3])
        sm1_finish(0, combo_0)

        # combo_1's f1b half loads now: its WAR (f1_1 readers) cleared at
        # P3-end, and it must not queue behind b1's amax all-reduce on
        # gpsimd (which can only run after P6)
        combo_1 = [kp.tile([P, 2 * N], bf16, name=f"combo_1{k}", tag=f"w{k}")
                   for k in range(NT)]
        for k in range(NT):
            nc.gpsimd.dma_start(combo_1[k][:, N:2 * N],
                                f1b_d[1, k * P:(k + 1) * P, :])

        # P5: b0.G3 first 5 m-tiles; rotation chained off P4's (4,0,2)
        for m in range(0, 5):
            g3_group(0, combo_0, m, psbase=(2, 4, 0)[m % 3])

        # P6: b1.G2
        e2t_1 = bigp.tile([P, BIG], bf16, name="e2t_1", tag="e2t")
        tr_last_1 = g2(1, f2s_1, e2t_1)

        # P7: b0.G3 tail + b1.G4, with b1's softmax-1 woven per-group;
        # b1's m7 transposes slot in behind the first tail group
        sm1_start(1)
        weave = iter(range(NT))
        first_tail = True
        for m in range(5, NT):
            g3_group(0, combo_0, m)
            if first_tail:
                tr_last_1()
                first_tail = False
            for wm in (next(weave, None), next(weave, None)):
                if wm is not None:
                    sm1_piece(1, wm, combo_1)
        for m in range(NT):
            g4_group(1, e2t_1, f2bs_1, m)
            wm = next(weave, None)
            if wm is not None:
                sm1_piece(1, wm, combo_1)
        sm1_finish(1, combo_1)

        # P8: b1.G3 — rotate PSUM banks [6,4,0,2]: banks 6/7 are free after
        # b1's transposes, so the first group never WAR-waits on b1.G4's
        # trailing drains, and the 4-deep cycle gives extra drain grace
        for m in range(NT):
            g3_group(1, combo_1, m, psbase=(6, 4, 0, 2)[m % 4],
                     last=(m == NT - 1))

    nc.compile()
    return nc


_NC = None
TRACE = False
LAST = None


def _get_nc():
    global _NC
    if _NC is None:
        _NC = _build()
    return _NC


def kernel(f1_norm, f2_norm, corr_weights):
    f1_norm = np.ascontiguousarray(f1_norm, dtype=np.float32)
    f2_norm = np.ascontiguousarray(f2_norm, dtype=np.float32)
    w = np.ascontiguousarray(corr_weights, dtype=np.float32)
    B = f1_norm.shape[0]
    assert B == NB * NCORES

    f1t = np.ascontiguousarray(np.swapaxes(f1_norm, 1, 2))
    f2t = np.ascontiguousarray(np.swapaxes(f2_norm, 1, 2))
    f1b = f1t.astype(ml_dtypes.bfloat16)
    f2b = f2t.astype(ml_dtypes.bfloat16)
    identb = np.eye(P, dtype=ml_dtypes.bfloat16)

    nc = _get_nc()
    in_maps = [
        {"f1t": f1t[c * NB:(c + 1) * NB], "f2t": f2t[c * NB:(c + 1) * NB],
         "f1b": f1b[c * NB:(c + 1) * NB], "f2b": f2b[c * NB:(c + 1) * NB],
         "w": w, "identb": identb}
        for c in range(NCORES)
    ]
    res = run_bass_kernel_spmd(nc, in_maps, core_ids=list(range(NCORES)), trace=TRACE)
    global LAST
    LAST = res
    out1 = np.concatenate([res.results[c]["o1"] for c in range(NCORES)], axis=0)
    out2 = np.concatenate([res.results[c]["o2"] for c in range(NCORES)], axis=0)
    return out1, out2
